# revision 1
# baseline (speedup 1.0000x reference)
"""Trainium2 Bass kernel for nn_MessageFunctionForEvent (GNN message function).

Math: the reference is
    em  = W_e2m @ e_wv[b] + b_e2m          (per-node Linear on edge features)
    nw  = W_n2m @ h_w[b]  + b_n2m          (per-node Linear on node features)
    nv  = W_n2m @ h_v[b]  + b_n2m          (node-level, no n axis)
    msg = Wa @ em + Wb @ nw + (Wc @ nv + b_resize)[:, None]
which collapses (precomposing the tiny 128x128 weights on host) to
    msg[b, :, n] = A @ e_wv[b, :, n] + Bm @ h_w[b, :, n] + c[b]
with A = Wa@W_e2m, Bm = Wb@W_n2m, c[b] = Wa@b_e2m + Wb@b_n2m + Wc@nv[b] + b_resize.

Device kernel: stream e/h column chunks HBM->SBUF on the sync HWDGE ring,
two accumulated 128x128 fp32 matmuls per 500-col PSUM tile (all A-passes of
a chunk first — they only need e — then B-passes as h lands), bias-add via
VectorE tensor_scalar_add (PSUM->SBUF), outputs on the scalar HWDGE ring.
Sharding: batch axis (16 batches -> 2 per core), zero host re-layout.
"""

import sys

import numpy as np

try:
    from concourse import bacc, mybir
except ImportError:  # bare environment: fall back to the in-container repo
    sys.path.append("/opt/trn_rl_repo")
    from concourse import bacc, mybir
import concourse.tile as tile
from concourse.bass_utils import run_bass_kernel_spmd

B, F, N = 16, 128, 20000
NCORES = 8
BPC = B // NCORES          # batches per core
CH = 5000                  # columns per DMA chunk
NT = 500                   # columns per matmul (fits one 2KB fp32 PSUM bank)

_cached_nc = None


def _build():
    global _cached_nc
    if _cached_nc is not None:
        return _cached_nc
    f32 = mybir.dt.float32
    nc = bacc.Bacc("TRN2", target_bir_lowering=False, debug=False,
                   num_devices=NCORES)
    e_d = nc.dram_tensor("e_wv", (BPC, F, N), f32, kind="ExternalInput").ap()
    h_d = nc.dram_tensor("h_w", (BPC, F, N), f32, kind="ExternalInput").ap()
    at_d = nc.dram_tensor("at", (F, F), f32, kind="ExternalInput").ap()
    bt_d = nc.dram_tensor("bt", (F, F), f32, kind="ExternalInput").ap()
    c_d = nc.dram_tensor("c", (F, BPC), f32, kind="ExternalInput").ap()
    o_d = nc.dram_tensor("msg", (BPC, F, N), f32, kind="ExternalOutput").ap()

    # chunk schedule per batch: big streaming chunks, tapered at the very
    # end so the pipeline drains with small PE/DMA quanta instead of one
    # full-size chunk of latency.
    def chunks_for(b):
        if b < BPC - 1:
            return [CH] * (N // CH)
        taper = [1000, 500, 500]
        return [CH] * (N // CH - 1) + [CH - sum(taper)] + taper

    with tile.TileContext(nc) as tc:
        with tc.tile_pool(name="w", bufs=1) as wp, \
             tc.tile_pool(name="eh", bufs=3) as ehp, \
             tc.tile_pool(name="out", bufs=3) as opp, \
             tc.tile_pool(name="ps", bufs=8, space="PSUM") as psp:
            at_t = wp.tile([F, F], f32)
            nc.gpsimd.dma_start(at_t[:], at_d[:])
            bt_t = wp.tile([F, F], f32)
            nc.gpsimd.dma_start(bt_t[:], bt_d[:])
            c_t = wp.tile([F, BPC], f32)
            nc.gpsimd.dma_start(c_t[:], c_d[:])
            for b in range(BPC):
                n0 = 0
                for cj, cs in enumerate(chunks_for(b)):
                    sl = slice(n0, n0 + cs)
                    e_t = ehp.tile([F, cs], f32, tag="e")
                    h_t = ehp.tile([F, cs], f32, tag="h")
                    o_t = opp.tile([F, cs], f32, tag="o")
                    nc.sync.dma_start(e_t[:], e_d[b, :, sl])
                    nc.sync.dma_start(h_t[:], h_d[b, :, sl])
                    nk = cs // NT if cs >= NT else 1
                    nt = cs // nk
                    ps_ts = []
                    for k in range(nk):
                        ksl = slice(k * nt, (k + 1) * nt)
                        ps_t = psp.tile([F, nt], f32, tag="ps")
                        ps_ts.append(ps_t)
                        nc.tensor.matmul(ps_t[:], at_t[:], e_t[:, ksl],
                                         start=True, stop=False)
                    for k in range(nk):
                        ksl = slice(k * nt, (k + 1) * nt)
                        nc.tensor.matmul(ps_ts[k][:], bt_t[:], h_t[:, ksl],
                                         start=False, stop=True)
                        nc.vector.tensor_scalar_add(o_t[:, ksl], ps_ts[k][:],
                                                    c_t[:, b:b + 1])
                        # stream the first half of the chunk out as soon as
                        # its bias-adds are done (halves drain latency)
                        if nk >= 4 and k == nk // 2 - 1:
                            nc.scalar.dma_start(o_d[b, :, n0:n0 + nt * (nk // 2)],
                                                o_t[:, :nt * (nk // 2)])
                    lo = nt * (nk // 2) if nk >= 4 else 0
                    nc.scalar.dma_start(o_d[b, :, n0 + lo:n0 + cs],
                                        o_t[:, lo:])
                    n0 += cs
    nc.finalize()
    _cached_nc = nc
    return nc


def _prepare_in_maps(h_w, h_v, e_wv, W_e2m, b_e2m, W_n2m, b_n2m,
                     W_resize, b_resize):
    f64 = np.float64
    M = F
    Wa = W_resize[:, :M].astype(f64)
    Wb = W_resize[:, M:2 * M].astype(f64)
    Wc = W_resize[:, 2 * M:].astype(f64)
    A = Wa @ W_e2m.astype(f64)
    Bm = Wb @ W_n2m.astype(f64)
    nv = h_v.astype(f64) @ W_n2m.astype(f64).T + b_n2m.astype(f64)
    c = (Wa @ b_e2m.astype(f64) + Wb @ b_n2m.astype(f64)
         + nv @ Wc.T + b_resize.astype(f64))          # [B, M]
    AT = np.ascontiguousarray(A.T).astype(np.float32)
    BT = np.ascontiguousarray(Bm.T).astype(np.float32)
    cT = np.ascontiguousarray(c.T).astype(np.float32)  # [M, B]

    in_maps = []
    for cid in range(NCORES):
        bs = slice(cid * BPC, (cid + 1) * BPC)
        in_maps.append({
            "e_wv": np.ascontiguousarray(e_wv[bs]),
            "h_w": np.ascontiguousarray(h_w[bs]),
            "at": AT,
            "bt": BT,
            "c": np.ascontiguousarray(cT[:, bs]),
        })
    return in_maps


def kernel(**inputs):
    args = {k: np.asarray(inputs[k], dtype=np.float32)
            for k in ("h_w", "h_v", "e_wv", "W_e2m", "b_e2m", "W_n2m",
                      "b_n2m", "W_resize", "b_resize")}
    in_maps = _prepare_in_maps(**args)
    nc = _build()
    res = run_bass_kernel_spmd(nc, in_maps, core_ids=list(range(NCORES)))
    return np.concatenate([r["msg"] for r in res.results], axis=0)



# revision 2
# speedup vs baseline: 1.8860x; 1.8860x over previous
"""Trainium2 Bass kernel for nn_MessageFunctionForEvent (GNN message function).

Math: the reference is
    em  = W_e2m @ e_wv[b] + b_e2m          (per-node Linear on edge features)
    nw  = W_n2m @ h_w[b]  + b_n2m          (per-node Linear on node features)
    nv  = W_n2m @ h_v[b]  + b_n2m          (node-level, no n axis)
    msg = Wa @ em + Wb @ nw + (Wc @ nv + b_resize)[:, None]
which collapses (precomposing the tiny 128x128 weights on host) to
    msg[b, :, n] = A @ e_wv[b, :, n] + Bm @ h_w[b, :, n] + c[b]
with A = Wa@W_e2m, Bm = Wb@W_n2m, c[b] = Wa@b_e2m + Wb@b_n2m + Wc@nv[b] + b_resize.

The problem is HBM-bound (per-core traffic >> compute), so the streams are
cast to bf16 on the host: e/h chunks and the two 128x128 weights go over the
wire in bf16, matmuls accumulate in fp32 PSUM, the bias-add writes a bf16
output tile, and the host upcasts the result to fp32. This halves HBM traffic
(61.4MB -> 30.7MB per core) for ~1.3e-3 normed rel error (gate is 2e-2).

Device kernel: stream e/h column chunks HBM->SBUF on the sync HWDGE ring,
two accumulated 128x128 matmuls per 500-col fp32 PSUM bank (all A-passes of
a chunk first — they only need e — then B-passes as h lands), bias-add via
VectorE tensor_scalar_add (PSUM fp32 -> SBUF bf16), outputs on the scalar
HWDGE ring. Sharding: batch axis (16 batches -> 2 per core).
"""

import sys

import numpy as np
import ml_dtypes

try:
    from concourse import bacc, mybir
except ImportError:  # bare environment: fall back to the in-container repo
    sys.path.append("/opt/trn_rl_repo")
    from concourse import bacc, mybir
import concourse.tile as tile
from concourse.bass_utils import run_bass_kernel_spmd

B, F, N = 16, 128, 20000
NCORES = 8
BPC = B // NCORES          # batches per core
CH = 4000                  # columns per DMA chunk
NT = 500                   # columns per matmul (fits one 2KB fp32 PSUM bank)
BF16 = np.dtype(ml_dtypes.bfloat16)

_cached_nc = None


def _build():
    global _cached_nc
    if _cached_nc is not None:
        return _cached_nc
    f32 = mybir.dt.float32
    bf16 = mybir.dt.bfloat16
    nc = bacc.Bacc("TRN2", target_bir_lowering=False, debug=False,
                   num_devices=NCORES)
    e_d = nc.dram_tensor("e_wv", (BPC, F, N), bf16, kind="ExternalInput").ap()
    h_d = nc.dram_tensor("h_w", (BPC, F, N), bf16, kind="ExternalInput").ap()
    at_d = nc.dram_tensor("at", (F, F), bf16, kind="ExternalInput").ap()
    bt_d = nc.dram_tensor("bt", (F, F), bf16, kind="ExternalInput").ap()
    c_d = nc.dram_tensor("c", (F, BPC), f32, kind="ExternalInput").ap()
    o_d = nc.dram_tensor("msg", (BPC, F, N), bf16, kind="ExternalOutput").ap()

    # chunk schedule per batch: big streaming chunks, tapered at the very
    # end so the pipeline drains with small PE/DMA quanta instead of one
    # full-size chunk of latency.
    def chunks_for(b):
        if b < BPC - 1:
            return [CH] * (N // CH)
        taper = [1000, 500, 500]
        return [CH] * (N // CH - 1) + [CH - sum(taper)] + taper

    with tile.TileContext(nc) as tc:
        with tc.tile_pool(name="w", bufs=1) as wp, \
             tc.tile_pool(name="eh", bufs=4) as ehp, \
             tc.tile_pool(name="out", bufs=4) as opp, \
             tc.tile_pool(name="ps", bufs=8, space="PSUM") as psp:
            at_t = wp.tile([F, F], bf16)
            nc.gpsimd.dma_start(at_t[:], at_d[:])
            bt_t = wp.tile([F, F], bf16)
            nc.gpsimd.dma_start(bt_t[:], bt_d[:])
            c_t = wp.tile([F, BPC], f32)
            nc.gpsimd.dma_start(c_t[:], c_d[:])
            for b in range(BPC):
                n0 = 0
                for cj, cs in enumerate(chunks_for(b)):
                    sl = slice(n0, n0 + cs)
                    e_t = ehp.tile([F, cs], bf16, tag="e")
                    h_t = ehp.tile([F, cs], bf16, tag="h")
                    o_t = opp.tile([F, cs], bf16, tag="o")
                    nc.sync.dma_start(e_t[:], e_d[b, :, sl])
                    nc.sync.dma_start(h_t[:], h_d[b, :, sl])
                    nk = cs // NT if cs >= NT else 1
                    nt = cs // nk
                    ps_ts = []
                    for k in range(nk):
                        ksl = slice(k * nt, (k + 1) * nt)
                        ps_t = psp.tile([F, nt], f32, tag="ps")
                        ps_ts.append(ps_t)
                        nc.tensor.matmul(ps_t[:], at_t[:], e_t[:, ksl],
                                         start=True, stop=False)
                    for k in range(nk):
                        ksl = slice(k * nt, (k + 1) * nt)
                        nc.tensor.matmul(ps_ts[k][:], bt_t[:], h_t[:, ksl],
                                         start=False, stop=True)
                        nc.vector.tensor_scalar_add(o_t[:, ksl], ps_ts[k][:],
                                                    c_t[:, b:b + 1])
                        # stream the first half of the chunk out as soon as
                        # its bias-adds are done (halves drain latency)
                        if nk >= 4 and k == nk // 2 - 1:
                            nc.scalar.dma_start(o_d[b, :, n0:n0 + nt * (nk // 2)],
                                                o_t[:, :nt * (nk // 2)])
                    lo = nt * (nk // 2) if nk >= 4 else 0
                    nc.scalar.dma_start(o_d[b, :, n0 + lo:n0 + cs],
                                        o_t[:, lo:])
                    n0 += cs
    nc.finalize()
    _cached_nc = nc
    return nc


def _prepare_in_maps(h_w, h_v, e_wv, W_e2m, b_e2m, W_n2m, b_n2m,
                     W_resize, b_resize):
    f64 = np.float64
    M = F
    Wa = W_resize[:, :M].astype(f64)
    Wb = W_resize[:, M:2 * M].astype(f64)
    Wc = W_resize[:, 2 * M:].astype(f64)
    A = Wa @ W_e2m.astype(f64)
    Bm = Wb @ W_n2m.astype(f64)
    nv = h_v.astype(f64) @ W_n2m.astype(f64).T + b_n2m.astype(f64)
    c = (Wa @ b_e2m.astype(f64) + Wb @ b_n2m.astype(f64)
         + nv @ Wc.T + b_resize.astype(f64))          # [B, M]
    AT = np.ascontiguousarray(A.T).astype(BF16)
    BT = np.ascontiguousarray(Bm.T).astype(BF16)
    cT = np.ascontiguousarray(c.T).astype(np.float32)  # [M, B]

    e_bf = e_wv.astype(BF16)
    h_bf = h_w.astype(BF16)
    in_maps = []
    for cid in range(NCORES):
        bs = slice(cid * BPC, (cid + 1) * BPC)
        in_maps.append({
            "e_wv": np.ascontiguousarray(e_bf[bs]),
            "h_w": np.ascontiguousarray(h_bf[bs]),
            "at": AT,
            "bt": BT,
            "c": np.ascontiguousarray(cT[:, bs]),
        })
    return in_maps


def kernel(**inputs):
    args = {k: np.asarray(inputs[k], dtype=np.float32)
            for k in ("h_w", "h_v", "e_wv", "W_e2m", "b_e2m", "W_n2m",
                      "b_n2m", "W_resize", "b_resize")}
    in_maps = _prepare_in_maps(**args)
    nc = _build()
    res = run_bass_kernel_spmd(nc, in_maps, core_ids=list(range(NCORES)))
    return np.concatenate(
        [r["msg"].astype(np.float32) for r in res.results], axis=0)


# revision 5
# speedup vs baseline: 1.9289x; 1.0228x over previous
"""Trainium2 Bass kernel for nn_MessageFunctionForEvent (GNN message function).

Math: the reference is
    em  = W_e2m @ e_wv[b] + b_e2m          (per-node Linear on edge features)
    nw  = W_n2m @ h_w[b]  + b_n2m          (per-node Linear on node features)
    nv  = W_n2m @ h_v[b]  + b_n2m          (node-level, no n axis)
    msg = Wa @ em + Wb @ nw + (Wc @ nv + b_resize)[:, None]
which collapses (precomposing the tiny 128x128 weights on host) to
    msg[b, :, n] = A @ e_wv[b, :, n] + Bm @ h_w[b, :, n] + c[b]
with A = Wa@W_e2m, Bm = Wb@W_n2m, c[b] = Wa@b_e2m + Wb@b_n2m + Wc@nv[b] + b_resize.

The problem is HBM-bound (per-core traffic >> compute), so the streams are
cast to bf16 on the host: e/h chunks and the two 128x128 weights go over the
wire in bf16, matmuls accumulate in fp32 PSUM, the bias-add writes a bf16
output tile, and the host upcasts the result to fp32. This halves HBM traffic
(61.4MB -> 30.7MB per core) for ~1.3e-3 normed rel error (gate is 2e-2).

Device kernel: a single HWDGE ring tops out ~385 GB/s but both rings
together sustain ~425 GB/s (the SBUF AXI fabric limit), so the two rings
are byte-balanced end-to-end: e chunks on the sync(SP) ring, h chunks on
the scalar(ACT) ring, and each chunk's output halves split across both
rings. The tiny precomposed weights go over the sync ring first (HWDGE,
~0.6us first-byte) so compute unblocks immediately. Two accumulated
128x128 matmuls per 500-col fp32 PSUM bank (all A-passes of a chunk
first — they only need e — then B-passes as h lands), bias-add via
VectorE tensor_scalar_add (PSUM fp32 -> SBUF bf16).
Sharding: batch axis (16 batches -> 2 per core).
"""

import sys

import numpy as np
import ml_dtypes

try:
    from concourse import bacc, mybir
except ImportError:  # bare environment: fall back to the in-container repo
    sys.path.append("/opt/trn_rl_repo")
    from concourse import bacc, mybir
import concourse.tile as tile
from concourse.bass_utils import run_bass_kernel_spmd

B, F, N = 16, 128, 20000
NCORES = 8
BPC = B // NCORES          # batches per core
CH = 4000                  # columns per DMA chunk
NT = 500                   # columns per matmul (fits one 2KB fp32 PSUM bank)
BF16 = np.dtype(ml_dtypes.bfloat16)

_cached_nc = None


def _build():
    global _cached_nc
    if _cached_nc is not None:
        return _cached_nc
    f32 = mybir.dt.float32
    bf16 = mybir.dt.bfloat16
    nc = bacc.Bacc("TRN2", target_bir_lowering=False, debug=False,
                   num_devices=NCORES)
    e_d = nc.dram_tensor("e_wv", (BPC, F, N), bf16, kind="ExternalInput").ap()
    h_d = nc.dram_tensor("h_w", (BPC, F, N), bf16, kind="ExternalInput").ap()
    at_d = nc.dram_tensor("at", (F, F), bf16, kind="ExternalInput").ap()
    bt_d = nc.dram_tensor("bt", (F, F), bf16, kind="ExternalInput").ap()
    c_d = nc.dram_tensor("c", (F, BPC), f32, kind="ExternalInput").ap()
    o_d = nc.dram_tensor("msg", (BPC, F, N), bf16, kind="ExternalOutput").ap()

    # chunk schedule per batch: big streaming chunks, tapered at the very
    # end so the pipeline drains with small PE/DMA quanta instead of one
    # full-size chunk of latency.
    def chunks_for(b):
        if b < BPC - 1:
            return [CH] * (N // CH)
        taper = [1000, 500, 500]
        return [CH] * (N // CH - 1) + [CH - sum(taper)] + taper

    with tile.TileContext(nc) as tc:
        with tc.tile_pool(name="w", bufs=1) as wp, \
             tc.tile_pool(name="eh", bufs=6) as ehp, \
             tc.tile_pool(name="out", bufs=4) as opp, \
             tc.tile_pool(name="ps", bufs=8, space="PSUM") as psp:
            at_t = wp.tile([F, F], bf16)
            nc.sync.dma_start(at_t[:], at_d[:])
            bt_t = wp.tile([F, F], bf16)
            nc.sync.dma_start(bt_t[:], bt_d[:])
            c_t = wp.tile([F, BPC], f32)
            nc.sync.dma_start(c_t[:], c_d[:])

            sched = []
            for b in range(BPC):
                n0 = 0
                for cs in chunks_for(b):
                    sched.append((b, n0, cs))
                    n0 += cs
            tiles = {}

            def load(i):
                b, n0, cs = sched[i]
                e_t = ehp.tile([F, cs], bf16, tag="e")
                h_t = ehp.tile([F, cs], bf16, tag="h")
                nc.sync.dma_start(e_t[:], e_d[b, :, n0:n0 + cs])
                nc.scalar.dma_start(h_t[:], h_d[b, :, n0:n0 + cs])
                tiles[i] = (e_t, h_t)

            def compute_store(i):
                b, n0, cs = sched[i]
                e_t, h_t = tiles.pop(i)
                o_t = opp.tile([F, cs], bf16, tag="o")
                nk = cs // NT if cs >= NT else 1
                nt = cs // nk
                ps_ts = []
                for k in range(nk):
                    ksl = slice(k * nt, (k + 1) * nt)
                    ps_t = psp.tile([F, nt], f32, tag="ps")
                    ps_ts.append(ps_t)
                    nc.tensor.matmul(ps_t[:], at_t[:], e_t[:, ksl],
                                     start=True, stop=False)
                for k in range(nk):
                    ksl = slice(k * nt, (k + 1) * nt)
                    nc.tensor.matmul(ps_ts[k][:], bt_t[:], h_t[:, ksl],
                                     start=False, stop=True)
                    nc.vector.tensor_scalar_add(o_t[:, ksl], ps_ts[k][:],
                                                c_t[:, b:b + 1])
                    # stream the first half of the chunk out as soon as its
                    # bias-adds are done (halves drain latency); the halves
                    # go to different rings to keep both byte-balanced
                    if nk >= 4 and k == nk // 2 - 1:
                        nc.sync.dma_start(o_d[b, :, n0:n0 + nt * (nk // 2)],
                                          o_t[:, :nt * (nk // 2)])
                lo = nt * (nk // 2) if nk >= 4 else 0
                ring = nc.scalar if nk >= 4 or i % 2 else nc.sync
                ring.dma_start(o_d[b, :, n0 + lo:n0 + cs], o_t[:, lo:])

            # software-pipelined trigger order: keep LOOK chunks of input
            # loads queued on each ring ahead of the compute/store triggers,
            # so an output trigger waiting on the DVE never starves the ring.
            LOOK = 3
            for i in range(len(sched)):
                load(i)
                if i >= LOOK:
                    compute_store(i - LOOK)
            for i in range(len(sched) - LOOK, len(sched)):
                compute_store(i)
    nc.finalize()
    _cached_nc = nc
    return nc


def _prepare_in_maps(h_w, h_v, e_wv, W_e2m, b_e2m, W_n2m, b_n2m,
                     W_resize, b_resize):
    f64 = np.float64
    M = F
    Wa = W_resize[:, :M].astype(f64)
    Wb = W_resize[:, M:2 * M].astype(f64)
    Wc = W_resize[:, 2 * M:].astype(f64)
    A = Wa @ W_e2m.astype(f64)
    Bm = Wb @ W_n2m.astype(f64)
    nv = h_v.astype(f64) @ W_n2m.astype(f64).T + b_n2m.astype(f64)
    c = (Wa @ b_e2m.astype(f64) + Wb @ b_n2m.astype(f64)
         + nv @ Wc.T + b_resize.astype(f64))          # [B, M]
    AT = np.ascontiguousarray(A.T).astype(BF16)
    BT = np.ascontiguousarray(Bm.T).astype(BF16)
    cT = np.ascontiguousarray(c.T).astype(np.float32)  # [M, B]

    e_bf = e_wv.astype(BF16)
    h_bf = h_w.astype(BF16)
    in_maps = []
    for cid in range(NCORES):
        bs = slice(cid * BPC, (cid + 1) * BPC)
        in_maps.append({
            "e_wv": np.ascontiguousarray(e_bf[bs]),
            "h_w": np.ascontiguousarray(h_bf[bs]),
            "at": AT,
            "bt": BT,
            "c": np.ascontiguousarray(cT[:, bs]),
        })
    return in_maps


def kernel(**inputs):
    args = {k: np.asarray(inputs[k], dtype=np.float32)
            for k in ("h_w", "h_v", "e_wv", "W_e2m", "b_e2m", "W_n2m",
                      "b_n2m", "W_resize", "b_resize")}
    in_maps = _prepare_in_maps(**args)
    nc = _build()
    res = run_bass_kernel_spmd(nc, in_maps, core_ids=list(range(NCORES)))
    return np.concatenate(
        [r["msg"].astype(np.float32) for r in res.results], axis=0)
